# revision 7
# baseline (speedup 1.0000x reference)
"""Trainium2 Bass kernel for nn_CausalSelfAttention_37417755083187.

Full-input contract: kernel(**inputs) takes the unsharded fp32 inputs and
returns the full [B, T, C] fp32 output.  Sharding: 8 cores = (2 batches) x
(4 head-groups of 4 heads).  Host ships x transposed [C, T] bf16, weights /
ve / cos / sin bf16.  Each core computes a partial projection output
(row-split Wproj); host sums 4 partials per batch.

v2 pipeline (vs v1): QKV runs as sequential per-tile gate/q/k/v chains
(2 PSUM banks instead of 4), attention score pairs get 2 banks x2, and PV +
out-proj share a 2-bank ring.  The schedule interleaves out-proj blocks and
next-chunk QKV chains into each attention section so the PE never idles on
the serial ACT exp chain.  RMS-norm rsqrt is computed entirely on DVE
(Quake seed + 2 Newton steps) so ACT keeps its exp table loaded; the
square+reduce is a single fused tensor_tensor_reduce.  Scores matmuls trim
the fully-masked query range of diagonal tiles.
"""

import os
import sys

sys.path.insert(0, "/opt/trn_rl_repo")

from contextlib import ExitStack

import numpy as np

import concourse.bass as bass
import concourse.mybir as mybir
import concourse.tile as tile
from concourse import bacc
from concourse.alu_op_type import AluOpType as alu

F32 = mybir.dt.float32
BF16 = mybir.dt.bfloat16
I32 = mybir.dt.int32
AF = mybir.ActivationFunctionType

B, T, C = 2, 2048, 2048
NH = 16
HD = 128
D2 = HD // 2
GATE = 32
EPS = 1e-6
N_CORES = 8
N_GROUPS = 4
NHC = NH // N_GROUPS
NQ = NHC * HD            # 512
TT = T // 128            # 16
CT = C // 128            # 16
NCH = T // 512           # 4
VW = 130                 # per-head v width: 128 v + 1 ones + 1 pad
QMAGIC = 0x5F3759DF + 1  # quake rsqrt magic (+1 folds the two's-complement)


def build_nc(num_devices=N_CORES):
    nc = bacc.Bacc(
        "TRN2",
        target_bir_lowering=False,
        debug=False,
        enable_asserts=False,
        num_devices=num_devices,
    )

    xT_d = nc.dram_tensor("xT_s", [C, T], BF16, kind="ExternalInput").ap()
    ve_d = nc.dram_tensor("ve_s", [T, NQ], BF16, kind="ExternalInput").ap()
    cos_d = nc.dram_tensor("cos_s", [T, D2], BF16, kind="ExternalInput").ap()
    sin_d = nc.dram_tensor("sin_s", [T, D2], BF16, kind="ExternalInput").ap()
    wq_d = nc.dram_tensor("wq_s", [C, NQ], BF16, kind="ExternalInput").ap()
    wk_d = nc.dram_tensor("wk_s", [C, NQ], BF16, kind="ExternalInput").ap()
    wv_d = nc.dram_tensor("wv_s", [C, NQ], BF16, kind="ExternalInput").ap()
    wg_d = nc.dram_tensor("wg_s", [GATE, NHC], BF16, kind="ExternalInput").ap()
    wp_d = nc.dram_tensor("wp_s", [NQ, C], BF16, kind="ExternalInput").ap()
    out_d = nc.dram_tensor("out_s", [T, C], F32, kind="ExternalOutput").ap()

    with ExitStack() as ctx:
        tc = ctx.enter_context(tile.TileContext(nc))
        pp = ctx.enter_context(tc.tile_pool(name="persist", bufs=1))
        pw = ctx.enter_context(tc.tile_pool(name="work", bufs=2))
        psS = ctx.enter_context(tc.tile_pool(name="psS", bufs=2, space="PSUM"))
        psA = ctx.enter_context(tc.tile_pool(name="psA", bufs=2, space="PSUM"))
        psY = ctx.enter_context(tc.tile_pool(name="psY", bufs=2, space="PSUM"))

        kT = pp.tile([128, NHC, T], BF16, name="kT")           # [d, h, t]
        vext = pp.tile([128, TT, NHC * VW], BF16, name="vext")
        g_all = pp.tile([128, TT, NHC], F32, name="g_all")
        cos_bf = pp.tile([128, TT, D2], BF16, name="cos_bf")
        sin_bf = pp.tile([128, TT, D2], BF16, name="sin_bf")
        wgate_b = pp.tile([GATE, NHC], BF16, name="wgate_b")
        wq_b = pp.tile([128, CT, NQ], BF16, name="wq_b")
        wk_b = pp.tile([128, CT, NQ], BF16, name="wk_b")
        wv_b = pp.tile([128, CT, NQ], BF16, name="wv_b")
        wp_b = pp.tile([128, NHC, C], BF16, name="wp_b")

        vext_v = vext.rearrange("p t (h c) -> p t h c", c=VW)
        xT_r = xT_d.rearrange("(a p) t -> p a t", p=128)

        # PE warmup keeps the HAM clock-gate warm through the DMA preamble.
        wu = pp.tile([128, 128], BF16, name="wu")
        nc.vector.memset(wu, 0.0)
        for _ in range(24):
            wu_ps = psA.tile([128, 512], F32, tag="a")
            nc.tensor.matmul(wu_ps[:, 0:128], wu, wu, start=True, stop=True)

        # ---- DMA preamble (scalar queue is FIFO: order = need order) ----
        nc.scalar.dma_start(wgate_b, wg_d)
        xTc_cache = {}
        xTc0 = pw.tile([128, CT, 256], BF16, tag="xT", bufs=2)
        nc.scalar.dma_start(xTc0, xT_r[:, :, 0:256])
        xTc_cache[0] = xTc0
        nq = CT // 4
        wq_r = wq_d.rearrange("(a p) n -> p a n", p=128)
        wk_r = wk_d.rearrange("(a p) n -> p a n", p=128)
        wv_r = wv_d.rearrange("(a p) n -> p a n", p=128)
        for qtr in range(4):
            nc.scalar.dma_start(wq_b[:, qtr * nq:(qtr + 1) * nq, :],
                                wq_r[:, qtr * nq:(qtr + 1) * nq, :])
        xTc1 = pw.tile([128, CT, 256], BF16, tag="xT", bufs=2)
        nc.scalar.dma_start(xTc1, xT_r[:, :, 256:512])
        xTc_cache[1] = xTc1
        nc.scalar.dma_start(cos_bf, cos_d.rearrange("(a p) d -> p a d", p=128))
        nc.scalar.dma_start(sin_bf, sin_d.rearrange("(a p) d -> p a d", p=128))
        for qtr in range(4):
            nc.scalar.dma_start(wk_b[:, qtr * nq:(qtr + 1) * nq, :],
                                wk_r[:, qtr * nq:(qtr + 1) * nq, :])
        for qtr in range(4):
            nc.scalar.dma_start(wv_b[:, qtr * nq:(qtr + 1) * nq, :],
                                wv_r[:, qtr * nq:(qtr + 1) * nq, :])
        nc.gpsimd.memset(vext, 0.0)
        nc.gpsimd.memset(vext_v[:, :, :, 128:129], 1.0)

        qT_tiles = {}
        qk2_tiles = {}
        P_tiles = {}
        yn_tiles = {}
        yT_tiles = {}

        # ---------------- A-section chains ----------------
        def emit_gq(t):
            """Gate matmul + q chain for token tile t (+ xT/ve prefetch)."""
            ch_t, t4 = divmod(t, 4)
            if t4 == 0:
                qT_tiles[ch_t] = pw.tile([128, NHC, 512], BF16, tag="qT",
                                         bufs=2, name=f"qT_{ch_t}")
            if t % 2 == 0 and t // 2 not in xTc_cache:
                xTc = pw.tile([128, CT, 256], BF16, tag="xT", bufs=2,
                              name=f"xTc_{t//2}")
                nc.scalar.dma_start(xTc, xT_r[:, :, t * 128:t * 128 + 256])
                xTc_cache[t // 2] = xTc
            xTc = xTc_cache[t // 2]
            tsl = slice((t % 2) * 128, (t % 2) * 128 + 128)

            gps = psA.tile([128, 512], F32, tag="a")
            nc.tensor.matmul(gps[:, 0:NHC], xTc[0:GATE, 0, tsl], wgate_b,
                             start=True, stop=True)
            qps = psA.tile([128, 512], F32, tag="a")
            for c in range(CT):
                nc.tensor.matmul(qps, xTc[:, c, tsl], wq_b[:, c, :],
                                 start=(c == 0), stop=(c == CT - 1))

            # gate epilogue: 2*sigmoid(2u') = 1 + tanh(u') via odd series
            gu = pw.tile([128, NHC], F32, tag="gu", bufs=2)
            nc.vector.tensor_copy(gu, gps[:, 0:NHC])
            ga = pw.tile([128, NHC], F32, tag="ga", bufs=2)
            nc.vector.tensor_mul(ga, gu, gu)
            gb = pw.tile([128, NHC], F32, tag="gb", bufs=2)
            nc.vector.tensor_mul(gb, ga, gu)
            gc = pw.tile([128, NHC], F32, tag="gc", bufs=2)
            nc.vector.scalar_tensor_tensor(out=gc, in0=gb, scalar=-1.0 / 3.0,
                                           in1=gu, op0=alu.mult, op1=alu.add)
            ge = pw.tile([128, NHC], F32, tag="ge", bufs=2)
            nc.vector.tensor_mul(ge, ga, gb)
            gf = pw.tile([128, NHC], F32, tag="gf", bufs=2)
            nc.vector.scalar_tensor_tensor(out=gf, in0=ge, scalar=2.0 / 15.0,
                                           in1=gc, op0=alu.mult, op1=alu.add)
            nc.vector.tensor_scalar_add(g_all[:, t, :], gf, 1.0)

            qk2 = pw.tile([128, 2, NQ], BF16, tag="qk2", bufs=4,
                          name=f"qk2_{t}")
            qk2_tiles[t] = qk2
            nc.scalar.copy(qk2[:, 0, :], qps)
        emit_gq.vet = {}

        def emit_k(t):
            """k chain + RoPE + RMS-norm + q/k transposes for tile t."""
            ch_t, t4 = divmod(t, 4)
            xTc = xTc_cache[t // 2]
            tsl = slice((t % 2) * 128, (t % 2) * 128 + 128)
            vet = pw.tile([128, NQ], BF16, tag="ve", bufs=4, name=f"ve_{t}")
            nc.scalar.dma_start(vet, ve_d[bass.ts(t, 128), :])
            emit_gq.vet[t] = vet
            kps = psA.tile([128, 512], F32, tag="a")
            for c in range(CT):
                nc.tensor.matmul(kps, xTc[:, c, tsl], wk_b[:, c, :],
                                 start=(c == 0), stop=(c == CT - 1))
            qk2 = qk2_tiles[t]
            nc.scalar.copy(qk2[:, 1, :], kps)

            qk4 = qk2.rearrange("p a (h x d) -> p a h x d", h=NHC, x=2)
            z1 = qk4[:, :, :, 0, :]
            z2 = qk4[:, :, :, 1, :]
            cb = cos_bf[:, t, :].unsqueeze(1).unsqueeze(1) \
                .broadcast_to([128, 2, NHC, D2])
            sb = sin_bf[:, t, :].unsqueeze(1).unsqueeze(1) \
                .broadcast_to([128, 2, NHC, D2])
            rot = pw.tile([128, 2, NQ], BF16, tag="rot", bufs=2)
            rot4 = rot.rearrange("p a (h x d) -> p a h x d", h=NHC, x=2)
            t1 = pw.tile([128, 2, NHC, D2], BF16, tag="t1", bufs=2)
            t2 = pw.tile([128, 2, NHC, D2], BF16, tag="t2", bufs=2)
            nc.vector.tensor_mul(t1, z1, cb)
            nc.vector.tensor_mul(t2, z2, sb)
            nc.vector.tensor_add(rot4[:, :, :, 0, :], t1, t2)
            nc.vector.tensor_mul(t1, z2, cb)
            nc.vector.tensor_mul(t2, z1, sb)
            nc.vector.tensor_sub(rot4[:, :, :, 1, :], t1, t2)

            # fused square+reduce, then eps/mean folds:
            #   q: m = sum(rot^2) + HD*eps   (rsqrt then also divides by sqrt(HD))
            #   k: m = sum(rot^2)/HD + eps
            rot_h = rot.rearrange("p a (h d) -> p a h d", h=NHC)
            sq = pw.tile([128, 2, NHC, HD], BF16, tag="sq", bufs=2)
            sums = pw.tile([128, 2, NHC], F32, tag="sums", bufs=2)
            nc.vector.tensor_mul(sq, rot_h, rot_h)
            nc.vector.reduce_sum(sums, sq, axis=mybir.AxisListType.X)
            nc.vector.tensor_scalar_add(sums[:, 0, :], sums[:, 0, :],
                                        float(HD) * EPS)
            nc.vector.tensor_scalar(out=sums[:, 1, :], in0=sums[:, 1, :],
                                    scalar1=1.0 / HD, scalar2=EPS,
                                    op0=alu.mult, op1=alu.add)
            # rsqrt on DVE: quake seed + 2 Newton steps (no ACT table swap)
            sh = pw.tile([128, 2, NHC], I32, tag="sh", bufs=2)
            nc.vector.tensor_scalar(out=sh, in0=sums.bitcast(I32),
                                    scalar1=1, scalar2=-1,
                                    op0=alu.logical_shift_right,
                                    op1=alu.bitwise_xor)
            r0i = pw.tile([128, 2, NHC], I32, tag="r0i", bufs=2)
            nc.vector.tensor_scalar_add(r0i, sh, QMAGIC)
            r0 = r0i.bitcast(F32)
            n1 = pw.tile([128, 2, NHC], F32, tag="n1", bufs=2)
            for _ in range(2):
                nc.vector.tensor_mul(n1, r0, r0)
                nc.vector.tensor_mul(n1, n1, sums)
                nc.vector.tensor_scalar(out=n1, in0=n1, scalar1=-0.5,
                                        scalar2=1.5, op0=alu.mult, op1=alu.add)
                nc.vector.tensor_mul(r0, r0, n1)
            for a in range(2):
                for h in range(NHC):
                    sl = rot[:, a, bass.ts(h, HD)]
                    nc.vector.tensor_scalar_mul(sl, sl, r0[:, a, h:h + 1])
            nc.sync.dma_start_transpose(qT_tiles[ch_t][:, :, bass.ts(t4, 128)],
                                        rot[:, 0, :])
            nc.sync.dma_start_transpose(kT[:, :, bass.ts(t, 128)],
                                        rot[:, 1, :])

        def emit_v(t):
            """v chain + gated value-embedding epilogue for tile t."""
            xTc = xTc_cache[t // 2]
            tsl = slice((t % 2) * 128, (t % 2) * 128 + 128)
            vps = psA.tile([128, 512], F32, tag="a")
            for c in range(CT):
                nc.tensor.matmul(vps, xTc[:, c, tsl], wv_b[:, c, :],
                                 start=(c == 0), stop=(c == CT - 1))
            vtmp = pw.tile([128, NQ], BF16, tag="vtmp", bufs=2)
            nc.scalar.copy(vtmp, vps)
            vet = emit_gq.vet.pop(t)
            for h in range(NHC):
                nc.vector.scalar_tensor_tensor(
                    out=vext_v[:, t, h, 0:128],
                    in0=vet[:, bass.ts(h, 128)],
                    scalar=g_all[:, t, h:h + 1],
                    in1=vtmp[:, bass.ts(h, 128)],
                    op0=alu.mult, op1=alu.add)

        # ---------------- B-section: scores/exp and PV ----------------
        def emit_spair(ch, h, p):
            n_tk = 4 * (ch + 1)
            qT = qT_tiles[ch]
            if p == 0:
                P_tiles[(ch, h)] = pw.tile([128, TT, 512], BF16, tag="P",
                                           bufs=2, name=f"P_{ch}_{h}")
            P_all = P_tiles[(ch, h)]
            if True:
                s_ps = psS.tile([128, 2, 512], F32, tag="s")
                for s2 in (0, 1):
                    i = 2 * p + s2
                    joff = (i - 4 * ch) * 128 if i >= 4 * ch else 0
                    nc.tensor.matmul(
                        s_ps[:, s2, joff:],
                        kT[:, h, bass.ts(i, 128)],
                        qT[:, h, joff:],
                        start=True, stop=True)
                if p == n_tk // 2 - 1:
                    # last diagonal pair: tq < 256 entirely masked
                    nc.vector.memset(P_all[:, 2 * p:2 * p + 2, 0:256], 0.0)
                    nc.scalar.activation(P_all[:, 2 * p:2 * p + 2, 256:],
                                         s_ps[:, :, 256:], AF.Exp)
                    nc.gpsimd.affine_select(
                        out=P_all[:, 2 * p:2 * p + 2, 256:],
                        in_=P_all[:, 2 * p:2 * p + 2, 256:],
                        pattern=[[-128, 2], [1, 256]],
                        compare_op=alu.is_ge,
                        fill=0.0,
                        base=512 * ch + 256 - 128 * 2 * p,
                        channel_multiplier=-1)
                else:
                    nc.scalar.activation(P_all[:, 2 * p:2 * p + 2, :], s_ps,
                                         AF.Exp)
                    if p == n_tk // 2 - 2:
                        nc.gpsimd.affine_select(
                            out=P_all[:, 2 * p:2 * p + 2, :],
                            in_=P_all[:, 2 * p:2 * p + 2, :],
                            pattern=[[-128, 2], [1, 512]],
                            compare_op=alu.is_ge,
                            fill=0.0,
                            base=512 * ch - 128 * 2 * p,
                            channel_multiplier=-1)

        def emit_pv(ch, h, q4):
            tqt = 4 * ch + q4
            if q4 == 0 and h == 0:
                yn_tiles[ch] = pw.tile([128, 4, NQ], BF16, tag="yn", bufs=2,
                                       name=f"yn_{ch}")
            yn = yn_tiles[ch]
            P_all = P_tiles[(ch, h)]
            y_ps = psY.tile([128, 512], F32, tag="y")
            for i in range(tqt + 1):
                nc.tensor.matmul(
                    y_ps[:, 0:HD + 1],
                    P_all[:, i, bass.ts(q4, 128)],
                    vext_v[:, i, h, 0:HD + 1],
                    start=(i == 0), stop=(i == tqt))
            dr = pw.tile([128, 1], F32, tag="dr", bufs=2)
            nc.vector.reciprocal(dr, y_ps[:, HD:HD + 1])
            nc.vector.tensor_scalar_mul(yn[:, q4, bass.ts(h, HD)],
                                        y_ps[:, 0:HD], dr)

        def emit_pv4(ch, h):
            for q4 in range(4):
                emit_pv(ch, h, q4)

        # ---------------- C-section: transpose + out-proj ----------------
        def emit_yT(ch, t4):
            if t4 == 0:
                yT_tiles[ch] = pw.tile([128, NHC, 4, 128], BF16, tag="yTc",
                                       bufs=2, name=f"yT_{ch}")
            nc.sync.dma_start_transpose(yT_tiles[ch][:, :, t4, :],
                                        yn_tiles[ch][:, t4, :])

        def emit_projc(ch, t4, c4):
            t = ch * 4 + t4
            yT = yT_tiles[ch]
            o_ps = psY.tile([128, 512], F32, tag="y")
            for h in range(NHC):
                nc.tensor.matmul(o_ps, yT[:, h, t4, :],
                                 wp_b[:, h, bass.ts(c4, 512)],
                                 start=(h == 0), stop=(h == NHC - 1))
            ob = pw.tile([128, 512], F32, tag="ob", bufs=2)
            nc.vector.tensor_copy(ob, o_ps)
            nc.gpsimd.dma_start(
                out_d[bass.ts(t, 128), bass.ts(c4, 512)], ob)

        # ---------------- schedule ----------------
        # Preamble: all four q chains first (PE work per DMA byte while the
        # weight stream is still arriving), then k chains, then v chains.
        for t in range(4):
            emit_gq(t)
        for t in range(4):
            emit_k(t)
        for t in range(4):
            emit_v(t)

        def emit_pv4(ch, h):
            for q4 in range(4):
                emit_pv(ch, h, q4)

        # chunk 0 (2 score pairs per head): A(1) chains fill the exp latency
        emit_spair(0, 0, 0); emit_spair(0, 0, 1)
        emit_gq(4)
        emit_spair(0, 1, 0); emit_spair(0, 1, 1)
        emit_pv4(0, 0)
        emit_k(4)
        emit_spair(0, 2, 0); emit_spair(0, 2, 1)
        emit_pv4(0, 1)
        emit_v(4)
        emit_spair(0, 3, 0); emit_spair(0, 3, 1)
        emit_pv4(0, 2)
        emit_gq(5)
        emit_pv(0, 3, 0); emit_yT(0, 0)
        emit_pv(0, 3, 1); emit_yT(0, 1)
        emit_pv(0, 3, 2); emit_yT(0, 2)
        emit_pv(0, 3, 3); emit_yT(0, 3)
        emit_k(5); emit_v(5)
        nc.scalar.dma_start(wp_b, wp_d.rearrange("(h p) c -> p h c", p=128))
        emit_gq(6); emit_k(6); emit_v(6)
        emit_gq(7); emit_k(7); emit_v(7)

        # chunks 1-3: weave scores pairs at the ACT exp rate with proj /
        # PV / next-chunk QKV fill units so the PE queue always has work.
        def weave(ch, fills, tail_fills):
            """Interleave scores pairs with fill units at ~the ACT exp rate.

            Ordering constraints honored here:
              - pv(ch,h,*) is fully emitted before spair(ch,h+2,0)
                (P_all ring of 2)
              - all pv for heads 0-2 precede the pv(3)/yT drain, so every
                yn column is written before its transpose
            """
            NP = 2 * (ch + 1)                # score pairs per head
            jq = list(fills)                 # proj-block units
            pending = []                     # (h_src, pv unit) not yet emitted

            def pop_fill(h):
                # alternate J / pending-pv to spread DVE load
                if jq and (len(pending) == 0 or (len(jq) + len(pending)) % 2):
                    jq.pop(0)()
                elif pending:
                    pending.pop(0)[1]()
                elif jq:
                    jq.pop(0)()

            for h in range(NHC):
                while pending and pending[0][0] <= h - 2:
                    pending.pop(0)[1]()
                emit_spair(ch, h, 0)
                emit_spair(ch, h, 1)
                if h > 0:
                    pending.extend(
                        (h - 1, lambda h=h, q4=q4: emit_pv(ch, h - 1, q4))
                        for q4 in range(4))
                for p in range(2, NP):
                    pop_fill(h)
                    emit_spair(ch, h, p)
            while pending:
                pending.pop(0)[1]()
            tq = list(tail_fills)
            k = 0
            for q4 in range(4):
                emit_pv(ch, 3, q4)
                emit_yT(ch, q4)
                for _ in range(2):
                    if jq:
                        jq.pop(0)()
                    elif k < len(tq):
                        tq[k](); k += 1
            while jq:
                jq.pop(0)()
            while k < len(tq):
                tq[k](); k += 1

        weave(1,
              fills=[lambda: emit_projc(0, 0, 0), lambda: emit_projc(0, 0, 1),
                     lambda: emit_projc(0, 0, 2), lambda: emit_projc(0, 0, 3)],
              tail_fills=[lambda t4=t4, c4=c4: emit_projc(0, t4, c4)
                          for t4 in range(1, 4) for c4 in range(4)])
        emit_gq(8); emit_k(8); emit_v(8)
        emit_gq(9); emit_k(9); emit_v(9)
        emit_gq(10); emit_k(10); emit_v(10)
        emit_gq(11); emit_k(11); emit_v(11)

        weave(2,
              fills=[lambda: emit_projc(1, 0, 0), lambda: emit_projc(1, 0, 1),
                     lambda: emit_projc(1, 0, 2), lambda: emit_projc(1, 0, 3),
                     lambda: emit_projc(1, 1, 0), lambda: emit_projc(1, 1, 1),
                     lambda: emit_projc(1, 1, 2), lambda: emit_projc(1, 1, 3)],
              tail_fills=[lambda t4=t4, c4=c4: emit_projc(1, t4, c4)
                          for t4 in range(2, 4) for c4 in range(4)])
        emit_gq(12); emit_k(12); emit_v(12)
        emit_gq(13); emit_k(13); emit_v(13)
        emit_gq(14); emit_k(14); emit_v(14)
        emit_gq(15); emit_k(15); emit_v(15)

        weave(3,
              fills=[lambda t4=t4, c4=c4: emit_projc(2, t4, c4)
                     for t4 in range(3) for c4 in range(4)],
              tail_fills=(
                  [lambda c4=c4: emit_projc(2, 3, c4) for c4 in range(4)]
                  + [lambda t4=t4, c4=c4: emit_projc(3, t4, c4)
                     for t4 in range(4) for c4 in range(4)]))

    nc.compile()
    return nc


def shard_inputs(inputs):
    """Full fp32 inputs -> list of 8 per-core input maps (bf16 layout)."""
    import ml_dtypes

    bf16 = ml_dtypes.bfloat16
    x = np.asarray(inputs["x"], np.float32)
    ve = np.asarray(inputs["ve"], np.float32)
    cos = np.asarray(inputs["cos"], np.float32).reshape(T, D2)
    sin = np.asarray(inputs["sin"], np.float32).reshape(T, D2)
    wq = np.asarray(inputs["Wq"], np.float32)
    wk = np.asarray(inputs["Wk"], np.float32)
    wv = np.asarray(inputs["Wv"], np.float32)
    wg = np.asarray(inputs["Wgate"], np.float32)
    wp = np.asarray(inputs["Wproj"], np.float32)

    cos_b = cos.astype(bf16)
    sin_b = sin.astype(bf16)
    xT = [np.ascontiguousarray(x[b].T.astype(bf16)) for b in range(B)]
    maps = []
    for core in range(N_CORES):
        b, g = divmod(core, N_GROUPS)
        sl = slice(g * NQ, (g + 1) * NQ)
        maps.append({
            "xT_s": xT[b],
            "ve_s": np.ascontiguousarray(ve[b][:, sl].astype(bf16)),
            "cos_s": cos_b,
            "sin_s": sin_b,
            "wq_s": np.ascontiguousarray(wq[:, sl].astype(bf16)),
            "wk_s": np.ascontiguousarray(wk[:, sl].astype(bf16)),
            "wv_s": np.ascontiguousarray(wv[:, sl].astype(bf16)),
            "wg_s": np.ascontiguousarray(
                (wg[:, g * NHC:(g + 1) * NHC] * 0.5).astype(bf16)),
            "wp_s": np.ascontiguousarray(wp[sl, :].astype(bf16)),
        })
    return maps


_NC_CACHE = {}


def _get_nc():
    if "nc" not in _NC_CACHE:
        _NC_CACHE["nc"] = build_nc()
    return _NC_CACHE["nc"]


def kernel(**inputs) -> np.ndarray:
    from concourse.bass_utils import run_bass_kernel_spmd

    nc = _get_nc()
    in_maps = shard_inputs(inputs)
    res = run_bass_kernel_spmd(nc, in_maps, list(range(N_CORES)))
    out = np.zeros((B, T, C), np.float32)
    for core in range(N_CORES):
        b = core // N_GROUPS
        out[b] += res.results[core]["out_s"]
    return out


# revision 10
# speedup vs baseline: 1.0319x; 1.0319x over previous
"""Trainium2 Bass kernel for nn_CausalSelfAttention_37417755083187.

Full-input contract: kernel(**inputs) takes the unsharded fp32 inputs and
returns the full [B, T, C] fp32 output.  Sharding: 8 cores = (2 batches) x
(4 head-groups of 4 heads).  Host ships x transposed [C, T] bf16, weights /
ve / cos / sin bf16.  Each core computes a partial projection output
(row-split Wproj); host sums 4 partials per batch.

v2 pipeline (vs v1): QKV runs as sequential per-tile gate/q/k/v chains
(2 PSUM banks instead of 4), attention score pairs get 2 banks x2, and PV +
out-proj share a 2-bank ring.  The schedule interleaves out-proj blocks and
next-chunk QKV chains into each attention section so the PE never idles on
the serial ACT exp chain.  RMS-norm rsqrt is computed entirely on DVE
(Quake seed + 2 Newton steps) so ACT keeps its exp table loaded; the
square+reduce is a single fused tensor_tensor_reduce.  Scores matmuls trim
the fully-masked query range of diagonal tiles.
"""

import os
import sys

sys.path.insert(0, "/opt/trn_rl_repo")

from contextlib import ExitStack

import numpy as np

import concourse.bass as bass
import concourse.mybir as mybir
import concourse.tile as tile
from concourse import bacc
from concourse.alu_op_type import AluOpType as alu

F32 = mybir.dt.float32
BF16 = mybir.dt.bfloat16
I32 = mybir.dt.int32
AF = mybir.ActivationFunctionType

B, T, C = 2, 2048, 2048
NH = 16
HD = 128
D2 = HD // 2
GATE = 32
EPS = 1e-6
N_CORES = 8
N_GROUPS = 4
NHC = NH // N_GROUPS
NQ = NHC * HD            # 512
TT = T // 128            # 16
CT = C // 128            # 16
NCH = T // 512           # 4
VW = 130                 # per-head v width: 128 v + 1 ones + 1 pad
QMAGIC = 0x5F3759DF + 1  # quake rsqrt magic (+1 folds the two's-complement)


def build_nc(num_devices=N_CORES):
    nc = bacc.Bacc(
        "TRN2",
        target_bir_lowering=False,
        debug=False,
        enable_asserts=False,
        num_devices=num_devices,
    )

    xT_d = nc.dram_tensor("xT_s", [T // 256, 128, CT, 256], BF16, kind="ExternalInput").ap()
    ve_d = nc.dram_tensor("ve_s", [T, NQ], BF16, kind="ExternalInput").ap()
    cos_d = nc.dram_tensor("cos_s", [128, TT, D2], BF16, kind="ExternalInput").ap()
    sin_d = nc.dram_tensor("sin_s", [128, TT, D2], BF16, kind="ExternalInput").ap()
    wq_d = nc.dram_tensor("wq_s", [128, CT, NQ], BF16, kind="ExternalInput").ap()
    wk_d = nc.dram_tensor("wk_s", [128, CT, NQ], BF16, kind="ExternalInput").ap()
    wv_d = nc.dram_tensor("wv_s", [128, CT, NQ], BF16, kind="ExternalInput").ap()
    wg_d = nc.dram_tensor("wg_s", [GATE, NHC], BF16, kind="ExternalInput").ap()
    wp_d = nc.dram_tensor("wp_s", [128, NHC, C], BF16, kind="ExternalInput").ap()
    out_d = nc.dram_tensor("out_s", [T, C], F32, kind="ExternalOutput").ap()

    with ExitStack() as ctx:
        tc = ctx.enter_context(tile.TileContext(nc))
        pp = ctx.enter_context(tc.tile_pool(name="persist", bufs=1))
        pw = ctx.enter_context(tc.tile_pool(name="work", bufs=2))
        psS = ctx.enter_context(tc.tile_pool(name="psS", bufs=2, space="PSUM"))
        psA = ctx.enter_context(tc.tile_pool(name="psA", bufs=2, space="PSUM"))
        psY = ctx.enter_context(tc.tile_pool(name="psY", bufs=2, space="PSUM"))

        kT = pp.tile([128, NHC, T], BF16, name="kT")           # [d, h, t]
        vext = pp.tile([128, TT, NHC * VW], BF16, name="vext")
        g_all = pp.tile([128, TT, NHC], F32, name="g_all")
        cos_bf = pp.tile([128, TT, D2], BF16, name="cos_bf")
        sin_bf = pp.tile([128, TT, D2], BF16, name="sin_bf")
        wgate_b = pp.tile([GATE, NHC], BF16, name="wgate_b")
        wq_b = pp.tile([128, CT, NQ], BF16, name="wq_b")
        wk_b = pp.tile([128, CT, NQ], BF16, name="wk_b")
        wv_b = pp.tile([128, CT, NQ], BF16, name="wv_b")
        wp_b = pp.tile([128, NHC, C], BF16, name="wp_b")

        vext_v = vext.rearrange("p t (h c) -> p t h c", c=VW)

        # PE warmup keeps the HAM clock-gate warm through the DMA preamble.
        wu = pp.tile([128, 128], BF16, name="wu")
        nc.vector.memset(wu, 0.0)
        for _ in range(24):
            wu_ps = psA.tile([128, 512], F32, tag="a")
            nc.tensor.matmul(wu_ps[:, 0:128], wu, wu, start=True, stop=True)

        # ---- DMA preamble (scalar queue is FIFO: order = need order) ----
        nc.scalar.dma_start(wgate_b, wg_d)
        xTc_cache = {}
        xTc0 = pw.tile([128, CT, 256], BF16, tag="xT", bufs=2)
        nc.scalar.dma_start(xTc0, xT_d[0])
        xTc_cache[0] = xTc0
        nq = CT // 4
        for qtr in range(4):
            nc.scalar.dma_start(wq_b[:, qtr * nq:(qtr + 1) * nq, :],
                                wq_d[:, qtr * nq:(qtr + 1) * nq, :])
        xTc1 = pw.tile([128, CT, 256], BF16, tag="xT", bufs=2)
        nc.scalar.dma_start(xTc1, xT_d[1])
        xTc_cache[1] = xTc1
        nc.scalar.dma_start(cos_bf, cos_d)
        nc.scalar.dma_start(sin_bf, sin_d)
        for qtr in range(4):
            nc.scalar.dma_start(wk_b[:, qtr * nq:(qtr + 1) * nq, :],
                                wk_d[:, qtr * nq:(qtr + 1) * nq, :])
        for qtr in range(4):
            nc.scalar.dma_start(wv_b[:, qtr * nq:(qtr + 1) * nq, :],
                                wv_d[:, qtr * nq:(qtr + 1) * nq, :])
        nc.gpsimd.memset(vext, 0.0)
        nc.gpsimd.memset(vext_v[:, :, :, 128:129], 1.0)

        qT_tiles = {}
        qk2_tiles = {}
        P_tiles = {}
        yn_tiles = {}
        yT_tiles = {}

        # ---------------- A-section chains ----------------
        def emit_gq(t):
            """Gate matmul + q chain for token tile t (+ xT/ve prefetch)."""
            ch_t, t4 = divmod(t, 4)
            if t4 == 0:
                qT_tiles[ch_t] = pw.tile([128, NHC, 512], BF16, tag="qT",
                                         bufs=2, name=f"qT_{ch_t}")
            if t % 2 == 0 and t // 2 not in xTc_cache:
                xTc = pw.tile([128, CT, 256], BF16, tag="xT", bufs=2,
                              name=f"xTc_{t//2}")
                nc.scalar.dma_start(xTc, xT_d[t // 2])
                xTc_cache[t // 2] = xTc
            xTc = xTc_cache[t // 2]
            tsl = slice((t % 2) * 128, (t % 2) * 128 + 128)

            gps = psA.tile([128, 512], F32, tag="a")
            nc.tensor.matmul(gps[:, 0:NHC], xTc[0:GATE, 0, tsl], wgate_b,
                             start=True, stop=True)
            qps = psA.tile([128, 512], F32, tag="a")
            for c in range(CT):
                nc.tensor.matmul(qps, xTc[:, c, tsl], wq_b[:, c, :],
                                 start=(c == 0), stop=(c == CT - 1))

            # gate epilogue: 2*sigmoid(2u') = 1 + tanh(u') via odd series
            gu = pw.tile([128, NHC], F32, tag="gu", bufs=2)
            nc.vector.tensor_copy(gu, gps[:, 0:NHC])
            ga = pw.tile([128, NHC], F32, tag="ga", bufs=2)
            nc.vector.tensor_mul(ga, gu, gu)
            gb = pw.tile([128, NHC], F32, tag="gb", bufs=2)
            nc.vector.tensor_mul(gb, ga, gu)
            gc = pw.tile([128, NHC], F32, tag="gc", bufs=2)
            nc.vector.scalar_tensor_tensor(out=gc, in0=gb, scalar=-1.0 / 3.0,
                                           in1=gu, op0=alu.mult, op1=alu.add)
            ge = pw.tile([128, NHC], F32, tag="ge", bufs=2)
            nc.vector.tensor_mul(ge, ga, gb)
            gf = pw.tile([128, NHC], F32, tag="gf", bufs=2)
            nc.vector.scalar_tensor_tensor(out=gf, in0=ge, scalar=2.0 / 15.0,
                                           in1=gc, op0=alu.mult, op1=alu.add)
            nc.vector.tensor_scalar_add(g_all[:, t, :], gf, 1.0)

            qk2 = pw.tile([128, 2, NQ], BF16, tag="qk2", bufs=4,
                          name=f"qk2_{t}")
            qk2_tiles[t] = qk2
            nc.scalar.copy(qk2[:, 0, :], qps)
        emit_gq.vet = {}

        def emit_k(t):
            """k chain + RoPE + RMS-norm + q/k transposes for tile t."""
            ch_t, t4 = divmod(t, 4)
            xTc = xTc_cache[t // 2]
            tsl = slice((t % 2) * 128, (t % 2) * 128 + 128)
            vet = pw.tile([128, NQ], BF16, tag="ve", bufs=4, name=f"ve_{t}")
            nc.scalar.dma_start(vet, ve_d[bass.ts(t, 128), :])
            emit_gq.vet[t] = vet
            kps = psA.tile([128, 512], F32, tag="a")
            for c in range(CT):
                nc.tensor.matmul(kps, xTc[:, c, tsl], wk_b[:, c, :],
                                 start=(c == 0), stop=(c == CT - 1))
            qk2 = qk2_tiles[t]
            nc.scalar.copy(qk2[:, 1, :], kps)

            qk4 = qk2.rearrange("p a (h x d) -> p a h x d", h=NHC, x=2)
            z1 = qk4[:, :, :, 0, :]
            z2 = qk4[:, :, :, 1, :]
            cb = cos_bf[:, t, :].unsqueeze(1).unsqueeze(1) \
                .broadcast_to([128, 2, NHC, D2])
            sb = sin_bf[:, t, :].unsqueeze(1).unsqueeze(1) \
                .broadcast_to([128, 2, NHC, D2])
            rot = pw.tile([128, 2, NQ], BF16, tag="rot", bufs=2)
            rot4 = rot.rearrange("p a (h x d) -> p a h x d", h=NHC, x=2)
            t1 = pw.tile([128, 2, NHC, D2], BF16, tag="t1", bufs=2)
            t2 = pw.tile([128, 2, NHC, D2], BF16, tag="t2", bufs=2)
            nc.vector.tensor_mul(t1, z1, cb)
            nc.vector.tensor_mul(t2, z2, sb)
            nc.vector.tensor_add(rot4[:, :, :, 0, :], t1, t2)
            nc.vector.tensor_mul(t1, z2, cb)
            nc.vector.tensor_mul(t2, z1, sb)
            nc.vector.tensor_sub(rot4[:, :, :, 1, :], t1, t2)

            # fused square+reduce, then eps/mean folds:
            #   q: m = sum(rot^2) + HD*eps   (rsqrt then also divides by sqrt(HD))
            #   k: m = sum(rot^2)/HD + eps
            rot_h = rot.rearrange("p a (h d) -> p a h d", h=NHC)
            sq = pw.tile([128, 2, NHC, HD], BF16, tag="sq", bufs=2)
            sums = pw.tile([128, 2, NHC], F32, tag="sums", bufs=2)
            nc.vector.tensor_mul(sq, rot_h, rot_h)
            nc.vector.reduce_sum(sums, sq, axis=mybir.AxisListType.X)
            nc.vector.tensor_scalar_add(sums[:, 0, :], sums[:, 0, :],
                                        float(HD) * EPS)
            nc.vector.tensor_scalar(out=sums[:, 1, :], in0=sums[:, 1, :],
                                    scalar1=1.0 / HD, scalar2=EPS,
                                    op0=alu.mult, op1=alu.add)
            # rsqrt on DVE: quake seed + 2 Newton steps (no ACT table swap)
            sh = pw.tile([128, 2, NHC], I32, tag="sh", bufs=2)
            nc.vector.tensor_scalar(out=sh, in0=sums.bitcast(I32),
                                    scalar1=1, scalar2=-1,
                                    op0=alu.logical_shift_right,
                                    op1=alu.bitwise_xor)
            r0i = pw.tile([128, 2, NHC], I32, tag="r0i", bufs=2)
            nc.vector.tensor_scalar_add(r0i, sh, QMAGIC)
            r0 = r0i.bitcast(F32)
            n1 = pw.tile([128, 2, NHC], F32, tag="n1", bufs=2)
            for _ in range(2):
                nc.vector.tensor_mul(n1, r0, r0)
                nc.vector.tensor_mul(n1, n1, sums)
                nc.vector.tensor_scalar(out=n1, in0=n1, scalar1=-0.5,
                                        scalar2=1.5, op0=alu.mult, op1=alu.add)
                nc.vector.tensor_mul(r0, r0, n1)
            for a in range(2):
                for h in range(NHC):
                    sl = rot[:, a, bass.ts(h, HD)]
                    nc.vector.tensor_scalar_mul(sl, sl, r0[:, a, h:h + 1])
            nc.scalar.dma_start_transpose(qT_tiles[ch_t][:, :, bass.ts(t4, 128)],
                                          rot[:, 0, :])
            nc.sync.dma_start_transpose(kT[:, :, bass.ts(t, 128)],
                                        rot[:, 1, :])

        def emit_v(t):
            """v chain + gated value-embedding epilogue for tile t."""
            xTc = xTc_cache[t // 2]
            tsl = slice((t % 2) * 128, (t % 2) * 128 + 128)
            vps = psA.tile([128, 512], F32, tag="a")
            for c in range(CT):
                nc.tensor.matmul(vps, xTc[:, c, tsl], wv_b[:, c, :],
                                 start=(c == 0), stop=(c == CT - 1))
            vtmp = pw.tile([128, NQ], BF16, tag="vtmp", bufs=2)
            nc.scalar.copy(vtmp, vps)
            vet = emit_gq.vet.pop(t)
            for h in range(NHC):
                nc.vector.scalar_tensor_tensor(
                    out=vext_v[:, t, h, 0:128],
                    in0=vet[:, bass.ts(h, 128)],
                    scalar=g_all[:, t, h:h + 1],
                    in1=vtmp[:, bass.ts(h, 128)],
                    op0=alu.mult, op1=alu.add)

        # ---------------- B-section: scores/exp and PV ----------------
        def emit_spair(ch, h, p):
            n_tk = 4 * (ch + 1)
            qT = qT_tiles[ch]
            if p == 0:
                P_tiles[(ch, h)] = pw.tile([128, TT, 512], BF16, tag="P",
                                           bufs=2, name=f"P_{ch}_{h}")
            P_all = P_tiles[(ch, h)]
            if True:
                s_ps = psS.tile([128, 2, 512], F32, tag="s")
                for s2 in (0, 1):
                    i = 2 * p + s2
                    joff = (i - 4 * ch) * 128 if i >= 4 * ch else 0
                    nc.tensor.matmul(
                        s_ps[:, s2, joff:],
                        kT[:, h, bass.ts(i, 128)],
                        qT[:, h, joff:],
                        start=True, stop=True)
                if p == n_tk // 2 - 1:
                    # last diagonal pair: tq < 256 entirely masked
                    nc.vector.memset(P_all[:, 2 * p:2 * p + 2, 0:256], 0.0)
                    nc.scalar.activation(P_all[:, 2 * p:2 * p + 2, 256:],
                                         s_ps[:, :, 256:], AF.Exp)
                    nc.gpsimd.affine_select(
                        out=P_all[:, 2 * p:2 * p + 2, 256:],
                        in_=P_all[:, 2 * p:2 * p + 2, 256:],
                        pattern=[[-128, 2], [1, 256]],
                        compare_op=alu.is_ge,
                        fill=0.0,
                        base=512 * ch + 256 - 128 * 2 * p,
                        channel_multiplier=-1)
                else:
                    nc.scalar.activation(P_all[:, 2 * p:2 * p + 2, :], s_ps,
                                         AF.Exp)
                    if p == n_tk // 2 - 2:
                        nc.gpsimd.affine_select(
                            out=P_all[:, 2 * p:2 * p + 2, :],
                            in_=P_all[:, 2 * p:2 * p + 2, :],
                            pattern=[[-128, 2], [1, 512]],
                            compare_op=alu.is_ge,
                            fill=0.0,
                            base=512 * ch - 128 * 2 * p,
                            channel_multiplier=-1)

        def emit_pv(ch, h, q4):
            tqt = 4 * ch + q4
            if q4 == 0 and h == 0:
                yn_tiles[ch] = pw.tile([128, 4, NQ], BF16, tag="yn", bufs=2,
                                       name=f"yn_{ch}")
            yn = yn_tiles[ch]
            P_all = P_tiles[(ch, h)]
            y_ps = psY.tile([128, 512], F32, tag="y")
            for i in range(tqt + 1):
                nc.tensor.matmul(
                    y_ps[:, 0:HD + 1],
                    P_all[:, i, bass.ts(q4, 128)],
                    vext_v[:, i, h, 0:HD + 1],
                    start=(i == 0), stop=(i == tqt))
            dr = pw.tile([128, 1], F32, tag="dr", bufs=2)
            nc.vector.reciprocal(dr, y_ps[:, HD:HD + 1])
            nc.vector.tensor_scalar_mul(yn[:, q4, bass.ts(h, HD)],
                                        y_ps[:, 0:HD], dr)

        def emit_pv4(ch, h):
            for q4 in range(4):
                emit_pv(ch, h, q4)

        # ---------------- C-section: transpose + out-proj ----------------
        def emit_yT(ch, t4):
            if t4 == 0:
                yT_tiles[ch] = pw.tile([128, NHC, 4, 128], BF16, tag="yTc",
                                       bufs=2, name=f"yT_{ch}")
            nc.sync.dma_start_transpose(yT_tiles[ch][:, :, t4, :],
                                        yn_tiles[ch][:, t4, :])

        def emit_projc(ch, t4, c4):
            t = ch * 4 + t4
            yT = yT_tiles[ch]
            o_ps = psY.tile([128, 512], F32, tag="y")
            for h in range(NHC):
                nc.tensor.matmul(o_ps, yT[:, h, t4, :],
                                 wp_b[:, h, bass.ts(c4, 512)],
                                 start=(h == 0), stop=(h == NHC - 1))
            ob = pw.tile([128, 512], F32, tag="ob", bufs=3)
            nc.vector.tensor_copy(ob, o_ps)
            nc.gpsimd.dma_start(
                out_d[bass.ts(t, 128), bass.ts(c4, 512)], ob)

        # ---------------- schedule ----------------
        # Preamble: all four q chains first (PE work per DMA byte while the
        # weight stream is still arriving), then k chains, then v chains.
        for t in range(4):
            emit_gq(t)
        for t in range(4):
            emit_k(t)
        for t in range(4):
            emit_v(t)

        def emit_pv4(ch, h):
            for q4 in range(4):
                emit_pv(ch, h, q4)

        # chunk 0 (2 score pairs per head): A(1) chains fill the exp
        # latency; gq/k(4) first cover the preamble transpose drain
        emit_gq(4); emit_k(4)
        emit_spair(0, 0, 0); emit_spair(0, 0, 1)
        emit_v(4)
        emit_spair(0, 1, 0); emit_spair(0, 1, 1)
        emit_pv4(0, 0)
        emit_gq(5)
        emit_spair(0, 2, 0); emit_spair(0, 2, 1)
        emit_pv4(0, 1)
        emit_k(5)
        emit_spair(0, 3, 0); emit_spair(0, 3, 1)
        emit_pv4(0, 2)
        emit_v(5)
        emit_pv(0, 3, 0); emit_yT(0, 0)
        emit_pv(0, 3, 1); emit_yT(0, 1)
        emit_pv(0, 3, 2); emit_yT(0, 2)
        emit_pv(0, 3, 3); emit_yT(0, 3)
        nc.scalar.dma_start(wp_b, wp_d)
        emit_gq(6); emit_k(6); emit_v(6)
        emit_gq(7); emit_k(7); emit_v(7)

        # chunks 1-3: weave scores pairs at the ACT exp rate with proj /
        # PV / next-chunk QKV fill units so the PE queue always has work.
        def weave(ch, fills, tail_fills):
            """Interleave scores pairs with fill units at ~the ACT exp rate.

            Ordering constraints honored here:
              - pv(ch,h,*) is fully emitted before spair(ch,h+2,0)
                (P_all ring of 2)
              - all pv for heads 0-2 precede the pv(3)/yT drain, so every
                yn column is written before its transpose
            """
            NP = 2 * (ch + 1)                # score pairs per head
            jq = list(fills)                 # proj-block units
            pending = []                     # (h_src, pv unit) not yet emitted

            def pop_fill(h):
                # alternate J / pending-pv to spread DVE load
                if jq and (len(pending) == 0 or (len(jq) + len(pending)) % 2):
                    jq.pop(0)()
                elif pending:
                    pending.pop(0)[1]()
                elif jq:
                    jq.pop(0)()

            for h in range(NHC):
                while pending and pending[0][0] <= h - 2:
                    pending.pop(0)[1]()
                emit_spair(ch, h, 0)
                emit_spair(ch, h, 1)
                if h > 0:
                    pending.extend(
                        (h - 1, lambda h=h, q4=q4: emit_pv(ch, h - 1, q4))
                        for q4 in range(4))
                for p in range(2, NP):
                    pop_fill(h)
                    emit_spair(ch, h, p)
            while pending:
                pending.pop(0)[1]()
            tq = list(tail_fills)
            k = 0
            for q4 in range(4):
                emit_pv(ch, 3, q4)
                emit_yT(ch, q4)
                for _ in range(2):
                    if jq:
                        jq.pop(0)()
                    elif k < len(tq):
                        tq[k](); k += 1
            while jq:
                jq.pop(0)()
            while k < len(tq):
                tq[k](); k += 1

        weave(1,
              fills=[lambda: emit_projc(0, 0, 0), lambda: emit_projc(0, 0, 1),
                     lambda: emit_projc(0, 0, 2), lambda: emit_projc(0, 0, 3)],
              tail_fills=[lambda t4=t4, c4=c4: emit_projc(0, t4, c4)
                          for t4 in range(1, 4) for c4 in range(4)])
        emit_gq(8); emit_k(8); emit_v(8)
        emit_gq(9); emit_k(9); emit_v(9)
        emit_gq(10); emit_k(10); emit_v(10)
        emit_gq(11); emit_k(11); emit_v(11)

        weave(2,
              fills=[lambda: emit_projc(1, 0, 0), lambda: emit_projc(1, 0, 1),
                     lambda: emit_projc(1, 0, 2), lambda: emit_projc(1, 0, 3),
                     lambda: emit_projc(1, 1, 0), lambda: emit_projc(1, 1, 1),
                     lambda: emit_projc(1, 1, 2), lambda: emit_projc(1, 1, 3)],
              tail_fills=[lambda t4=t4, c4=c4: emit_projc(1, t4, c4)
                          for t4 in range(2, 4) for c4 in range(4)])
        emit_gq(12); emit_k(12); emit_v(12)
        emit_gq(13); emit_k(13); emit_v(13)
        emit_gq(14); emit_k(14); emit_v(14)
        emit_gq(15); emit_k(15); emit_v(15)

        weave(3,
              fills=[lambda t4=t4, c4=c4: emit_projc(2, t4, c4)
                     for t4 in range(3) for c4 in range(4)],
              tail_fills=(
                  [lambda c4=c4: emit_projc(2, 3, c4) for c4 in range(4)]
                  + [lambda t4=t4, c4=c4: emit_projc(3, t4, c4)
                     for t4 in range(4) for c4 in range(4)]))

    nc.compile()
    return nc


def shard_inputs(inputs):
    """Full fp32 inputs -> list of 8 per-core input maps (bf16 layout)."""
    import ml_dtypes

    bf16 = ml_dtypes.bfloat16
    x = np.asarray(inputs["x"], np.float32)
    ve = np.asarray(inputs["ve"], np.float32)
    cos = np.asarray(inputs["cos"], np.float32).reshape(T, D2)
    sin = np.asarray(inputs["sin"], np.float32).reshape(T, D2)
    wq = np.asarray(inputs["Wq"], np.float32)
    wk = np.asarray(inputs["Wk"], np.float32)
    wv = np.asarray(inputs["Wv"], np.float32)
    wg = np.asarray(inputs["Wgate"], np.float32)
    wp = np.asarray(inputs["Wproj"], np.float32)

    def w_pan(w):
        # [C, n] -> [p=128, a=C/128, n] contiguous (4KB quarter lines)
        n = w.shape[1]
        return np.ascontiguousarray(
            w.reshape(CT, 128, n).transpose(1, 0, 2).astype(bf16))

    cos_b = np.ascontiguousarray(
        cos.reshape(TT, 128, D2).transpose(1, 0, 2).astype(bf16))
    sin_b = np.ascontiguousarray(
        sin.reshape(TT, 128, D2).transpose(1, 0, 2).astype(bf16))
    # x^T [C, T] -> [tchunk=8, p=128, a=CT, 256]: each 256-token x-chunk
    # lands as one fully contiguous 1MB DMA
    xT = [np.ascontiguousarray(
        x[b].T.reshape(CT, 128, T // 256, 256).transpose(2, 1, 0, 3)
        .astype(bf16)) for b in range(B)]
    maps = []
    for core in range(N_CORES):
        b, g = divmod(core, N_GROUPS)
        sl = slice(g * NQ, (g + 1) * NQ)
        maps.append({
            "xT_s": xT[b],
            "ve_s": np.ascontiguousarray(ve[b][:, sl].astype(bf16)),
            "cos_s": cos_b,
            "sin_s": sin_b,
            "wq_s": w_pan(wq[:, sl]),
            "wk_s": w_pan(wk[:, sl]),
            "wv_s": w_pan(wv[:, sl]),
            "wg_s": np.ascontiguousarray(
                (wg[:, g * NHC:(g + 1) * NHC] * 0.5).astype(bf16)),
            "wp_s": np.ascontiguousarray(
                wp[sl, :].reshape(NHC, 128, C).transpose(1, 0, 2)
                .astype(bf16)),
        })
    return maps


_NC_CACHE = {}


def _get_nc():
    if "nc" not in _NC_CACHE:
        _NC_CACHE["nc"] = build_nc()
    return _NC_CACHE["nc"]


def kernel(**inputs) -> np.ndarray:
    from concourse.bass_utils import run_bass_kernel_spmd

    nc = _get_nc()
    in_maps = shard_inputs(inputs)
    res = run_bass_kernel_spmd(nc, in_maps, list(range(N_CORES)))
    out = np.zeros((B, T, C), np.float32)
    for core in range(N_CORES):
        b = core // N_GROUPS
        out[b] += res.results[core]["out_s"]
    return out
